# revision 10
# baseline (speedup 1.0000x reference)
"""Trainium2 Bass kernel for nn_EquiLinear_6708738916908.

Reference computation (BT=32, N_ATOMS=8192, N_CGS=512, KNN=16):
    dist_vec[b, i*K+k, e] = cg_xyz[b, k+1, e] - cg_xyz[b, i, e]
    dx_recon  = einsum('bje,nj->bne', dist_vec, B_param)          # [B, N, 3]
    cg_offset = einsum('bin,bij->bjn', dx_recon, assign_norm)     # [B, C, 3]
    xyz_recon = cg_xyz[:, idx] - cg_offset[:, idx] + dx_recon
    returns (soft_assign, xyz, xyz_recon)   # first two pass through

Key algebraic fold (exact): dist_vec is low-rank in cg_xyz, so
    dx_recon[b,n,e] = sum_c G[n,c] * cg_xyz[b,c,e]
with G[n,c] = (sum_i B[n, i*K + (c-1)] for 1<=c<=K) - sum_k B[n, c*K + k].
G is folded once on the host (float64 accumulate); the device reads G (16MB)
instead of B_param (256MB) and contracts over 512 instead of 8192.

Sharding: data-parallel over batch, 4 batches per core x 8 cores. Each core
reads its assign_norm slice (64MB, the dominant irreducible HBM traffic),
computes dx for all atoms of its batches and the full cg_offset for its
batches (contraction over ALL atoms is local since atoms aren't sharded).
No collectives. Device returns dx[8192, 12] and H = cg_xyz - cg_offset
[12, 512]; the host does the (tiny) 512-row gather + add epilogue.
"""

import sys

if "/opt/trn_rl_repo" not in sys.path:
    sys.path.insert(0, "/opt/trn_rl_repo")

import numpy as np

import concourse.bass as bass
import concourse.mybir as mybir
import concourse.tile as tile
from concourse.vector_clock import ScopedClock
from concourse.bass_utils import run_bass_kernel_spmd

BT, N_ATOMS, N_CGS, KNN = 32, 8192, 512, 16
N_CORES = 8
B_LOC = BT // N_CORES          # 4 batches per core
X = 3 * B_LOC                  # 12 fused (batch, xyz-component) columns
P = 128
NT = N_ATOMS // P              # 64 atom tiles
KT = N_CGS // P                # 4 cg tiles
GRP = 4                        # atom tiles per assign_norm DMA (1MB chunks)
NG = NT // GRP                 # 16 groups


def _split_multi_waits(nc):
    """This toolchain's walrus build encodes a single sem-wait slot per
    instruction and errors on more ("Too many sync wait commands"). Tile's
    scheduler attaches one wait per producer lane, so hoist all but the last
    wait of each instruction onto single-wait NOPs inserted just before it on
    the same engine — identical semantics, since the engine sequencer blocks
    on each wait in program order."""
    ctr = 0
    for f in nc.m.functions:
        for bb in f.blocks:
            old = bb.instructions
            new = []
            changed = False
            for inst in old:
                si = getattr(inst, "sync_info", None)
                if si is not None and si.on_update and len(si.on_update) > 1:
                    raise AssertionError(
                        f"multi-update instruction {inst.name}: unsupported"
                    )
                if si is not None and si.on_wait and len(si.on_wait) > 1:
                    waits = list(si.on_wait)
                    for w in waits[:-1]:
                        nop = mybir.InstNoOp(
                            name=f"splitwait_{ctr}", ins=[], outs=[]
                        )
                        ctr += 1
                        nop.engine = inst.engine
                        nop.sync_info = mybir.SyncInfo(
                            on_wait=[w], on_update=[]
                        )
                        new.append(nop)
                    si.on_wait = waits[-1:]
                    inst.sync_info = si
                    changed = True
                new.append(inst)
            if changed:
                bb.instructions = new
    # sanity: verify the rewrite survived rust-side serialization
    for f in nc.m.functions:
        for bb in f.blocks:
            for inst in bb.instructions:
                si = getattr(inst, "sync_info", None)
                assert si is None or not si.on_wait or len(si.on_wait) <= 1, (
                    f"multi-wait survived on {inst.name}"
                )


def _build_bass():
    f32 = mybir.dt.float32
    nc = bass.Bass("TRN2", target_bir_lowering=False, debug=False)
    an = nc.dram_tensor("an", [B_LOC, N_ATOMS, N_CGS], f32, kind="ExternalInput").ap()
    gt = nc.dram_tensor("gt", [N_CGS, N_ATOMS], f32, kind="ExternalInput").ap()
    cgk = nc.dram_tensor("cgk", [N_CGS, X], f32, kind="ExternalInput").ap()
    cgt = nc.dram_tensor("cgt", [X, N_CGS], f32, kind="ExternalInput").ap()
    dxout = nc.dram_tensor("dxout", [NG, P, GRP, X], f32, kind="ExternalOutput").ap()
    hout = nc.dram_tensor("hout", [X, N_CGS], f32, kind="ExternalOutput").ap()

    with tile.TileContext(nc) as tc:
        with (
            tc.tile_pool(name="consts", bufs=1) as consts,
            tc.tile_pool(name="gtp", bufs=3) as gtp,
            tc.tile_pool(name="anp", bufs=6) as anp,
            tc.tile_pool(name="dxp", bufs=3) as dxp,
            tc.tile_pool(name="hp", bufs=2) as hp,
            tc.tile_pool(name="ps_dx", bufs=2, space="PSUM") as ps_dx,
            tc.tile_pool(name="ps_cg", bufs=B_LOC, space="PSUM") as ps_cg,
        ):
            # cgk striped to [p, kt, x] so each kt slice is a [128, X] rhs
            cgk_sb = consts.tile([P, KT, X], f32)
            nc.sync.dma_start(cgk_sb[:], cgk.rearrange("(kt p) x -> p kt x", p=P))
            # cgt as [e, b, j]: each [:, b, :] is a base-partition-0 [3, 512]
            cgt_sb = consts.tile([3, B_LOC, N_CGS], f32)
            nc.sync.dma_start(cgt_sb[:], cgt.rearrange("(b e) j -> e b j", e=3))

            # cg_offset^T accumulators: [3(e), 512(j)] per local batch
            pscg = [
                ps_cg.tile([3, N_CGS], f32, tag="pscg", name=f"pscg{b}")
                for b in range(B_LOC)
            ]

            for g in range(NG):
                gt_t = gtp.tile([P, KT, GRP * P], f32)
                nc.sync.dma_start(
                    gt_t[:],
                    gt[:, g * GRP * P : (g + 1) * GRP * P].rearrange(
                        "(kt p) n -> p kt n", p=P
                    ),
                )
                an_ts = []
                for b in range(B_LOC):
                    an_t = anp.tile([P, GRP, N_CGS], f32, tag="an", name=f"an{g}_{b}")
                    nc.sync.dma_start(
                        an_t[:],
                        an[b, g * GRP * P : (g + 1) * GRP * P, :].rearrange(
                            "(s p) j -> p s j", p=P
                        ),
                    )
                    an_ts.append(an_t)

                dxg = dxp.tile([P, GRP, X], f32)
                for s in range(GRP):
                    it = g * GRP + s
                    psd = ps_dx.tile([P, X], f32, tag="psd", name=f"psd{it}")
                    for kt in range(KT):
                        nc.tensor.matmul(
                            psd[:],
                            gt_t[:, kt, s * P : (s + 1) * P],
                            cgk_sb[:, kt],
                            start=(kt == 0),
                            stop=(kt == KT - 1),
                        )
                    nc.vector.tensor_copy(dxg[:, s], psd[:])
                    for b in range(B_LOC):
                        nc.tensor.matmul(
                            pscg[b][:],
                            dxg[:, s, 3 * b : 3 * b + 3],
                            an_ts[b][:, s],
                            start=(it == 0),
                            stop=(it == NT - 1),
                        )
                nc.sync.dma_start(dxout[g], dxg[:])

            for b in range(B_LOC):
                h_b = hp.tile([3, N_CGS], f32, tag="h", name=f"h{b}")
                nc.vector.tensor_tensor(
                    h_b[:], cgt_sb[:, b], pscg[b][:], mybir.AluOpType.subtract
                )
                nc.sync.dma_start(hout[3 * b : 3 * b + 3, :], h_b[:])
    _split_multi_waits(nc)
    return nc


_NC_CACHE = None
_LAST_IN_MAPS = None


def _get_nc():
    global _NC_CACHE
    if _NC_CACHE is None:
        _NC_CACHE = _build_bass()
    return _NC_CACHE


def _fold_g(B_param: np.ndarray, knn: int) -> np.ndarray:
    """G[n,c] such that dx[b,n,e] = sum_c G[n,c] cg_xyz[b,c,e] (exact fold)."""
    Br = B_param.reshape(N_ATOMS, N_CGS, knn)
    Bi = Br.sum(axis=2, dtype=np.float64)          # [n, 512] sum over k
    Bk = Br.sum(axis=1, dtype=np.float64)          # [n, knn] sum over i
    G = -Bi
    G[:, 1 : knn + 1] += Bk
    return G.astype(np.float32)


def kernel(xyz, cg_xyz, assign_norm, soft_assign, B_param, assign_idx, knn):
    xyz = np.asarray(xyz, dtype=np.float32)
    cg_xyz = np.asarray(cg_xyz, dtype=np.float32)
    assign_norm = np.asarray(assign_norm, dtype=np.float32)
    soft_assign = np.asarray(soft_assign)
    B_param = np.asarray(B_param, dtype=np.float32)
    idx = np.asarray(assign_idx).astype(np.int64)
    knn = int(knn)

    G = _fold_g(B_param, knn)
    gt = np.ascontiguousarray(G.T)                                  # [512, 8192]

    in_maps = []
    for c in range(N_CORES):
        cg_c = cg_xyz[c * B_LOC : (c + 1) * B_LOC]                  # [4, 512, 3]
        in_maps.append(
            {
                "an": np.ascontiguousarray(
                    assign_norm[c * B_LOC : (c + 1) * B_LOC]
                ),
                "gt": gt,
                "cgk": np.ascontiguousarray(
                    cg_c.transpose(1, 0, 2).reshape(N_CGS, X)
                ),
                "cgt": np.ascontiguousarray(
                    cg_c.transpose(0, 2, 1).reshape(X, N_CGS)
                ),
            }
        )

    global _LAST_IN_MAPS
    _LAST_IN_MAPS = in_maps
    res = run_bass_kernel_spmd(_get_nc(), in_maps, core_ids=list(range(N_CORES)))

    xyz_recon = np.empty((BT, N_ATOMS, 3), dtype=np.float32)
    for c in range(N_CORES):
        out = res.results[c]
        # dxout[g, p, s, x]: atom n = g*512 + s*128 + p, column x = 3*b + e
        dx = (
            out["dxout"].transpose(0, 2, 1, 3).reshape(N_ATOMS, B_LOC, 3)
        )                                                            # [n, b, e]
        H = out["hout"].reshape(B_LOC, 3, N_CGS)                     # [b, e, j]
        # xyz_recon[b, n, e] = dx[n, b, e] + H[b, e, idx[n]]
        xyz_recon[c * B_LOC : (c + 1) * B_LOC] = dx.transpose(1, 0, 2) + H[
            :, :, idx
        ].transpose(0, 2, 1)

    return (soft_assign, xyz, xyz_recon)
